# revision 1
# baseline (speedup 1.0000x reference)
"""GAT layer (B=8, N=2048, F=64) on 8 trn2 NeuronCores.

Strategy: data-parallel over batch B — one graph per core, adj replicated.
The device does all O(N^2) work: score = leaky_relu(e1_i + e2_j), p =
exp(score) * adj, out_i = (p @ Wh)_i / sum_j p_ij, elu.  Softmax max-
subtraction is skipped (scores are O(1)-bounded) and normalization is
folded into a cheap post-matmul per-row divide, so the N x N attention
matrix is never normalized elementwise.

Host-side prep (cheap O(N*F) / layout work):
  - Wh = h @ W^T, e1 = Wh@a1, e2 = Wh@a2 (the exact decomposition the
    reference uses), laid out for the device.
  - adj is transposed so the device's [j-partition, i-free] attention
    orientation streams adj rows contiguously, and cast to bf16 (adj is
    0/1-valued: exact in bf16, halves the HBM traffic).

Device layout (per core): scores live as [128 j x 2048 i] tiles (16 j-tiles).
  E1[p, i] = e1[i] broadcast over partitions (DMA partition-broadcast),
  e2 per-partition scalars.  ACT does Lrelu (alpha folded) + Exp (both in
  the exp_and_others table set -> one table load); DVE applies the adj
  mask; PE accumulates [Wh | 1]^T @ p into PSUM over all 16 j-tiles giving
  outT[65, 2048] (row 64 = softmax denominators).  Epilogue PE-transposes
  65x128 chunks so division + elu happen on tiny [128, 64] tiles and the
  output DMAs out in natural [N, F] orientation.
"""

import sys

import numpy as np
import ml_dtypes

for _p in ("/opt/trn_rl_repo",):
    if _p not in sys.path:
        sys.path.insert(0, _p)

from contextlib import ExitStack

import concourse.bass as bass
import concourse.tile as tile
from concourse import bacc, mybir
from concourse.bass_utils import run_bass_kernel_spmd
from concourse.masks import make_identity

B, N, F = 8, 2048, 64
P = 128
T = N // P  # 16 j-tiles
ALPHA = 0.2
NB = N // 512  # 4 psum banks of moving-free 512

_CACHE = {}


def _build_program():
    if "nc" in _CACHE:
        return _CACHE["nc"]
    dt = mybir.dt
    nc = bacc.Bacc("TRN2", target_bir_lowering=False, debug=False)

    adjT = nc.dram_tensor("adjT", [N, N], dt.float16, kind="ExternalInput").ap()
    whg = nc.dram_tensor("whg", [P, T * 65], dt.float16, kind="ExternalInput").ap()
    gr = nc.dram_tensor("gr", [1, N], dt.float16, kind="ExternalInput").ap()
    a2b = nc.dram_tensor("a2b", [P, T], dt.float32, kind="ExternalInput").ap()
    b2b = nc.dram_tensor("b2b", [P, T], dt.float32, kind="ExternalInput").ap()
    out = nc.dram_tensor("out", [N, F], dt.float32, kind="ExternalOutput").ap()

    with tile.TileContext(nc) as tc, ExitStack() as ctx:
        singles = ctx.enter_context(tc.tile_pool(name="singles", bufs=1))
        adjp = ctx.enter_context(tc.tile_pool(name="adjp", bufs=6))
        work = ctx.enter_context(tc.tile_pool(name="work", bufs=3))
        accp = ctx.enter_context(tc.tile_pool(name="accp", bufs=1, space="PSUM"))
        ptmp = ctx.enter_context(tc.tile_pool(name="ptmp", bufs=4, space="PSUM"))
        outp = ctx.enter_context(tc.tile_pool(name="outp", bufs=8))

        a2_sb = singles.tile([P, T], dt.float32)
        nc.scalar.dma_start(out=a2_sb[:], in_=a2b)
        b2_sb = singles.tile([P, T], dt.float32)
        nc.scalar.dma_start(out=b2_sb[:], in_=b2b)
        gr_sb = singles.tile([1, N], dt.float16)
        nc.scalar.dma_start(out=gr_sb[:], in_=gr)
        ones_sb = singles.tile([1, P], dt.float16)
        nc.vector.memset(ones_sb[:], 1.0)
        whg_sb = singles.tile([P, T * 65], dt.float16)
        nc.scalar.dma_start(out=whg_sb[:], in_=whg)
        ident = singles.tile([65, 65], dt.float32)
        make_identity(nc, ident[:])
        # G[p, i] = exp(0.8*e1_i) broadcast over partitions, via K=1 matmuls
        G = singles.tile([P, N], dt.float16)
        for n in range(NB):
            gb = ptmp.tile([P, 512], dt.float32, tag="tr", name=f"gb{n}")
            nc.tensor.matmul(
                out=gb[:],
                lhsT=ones_sb[:],
                rhs=gr_sb[:, n * 512 : (n + 1) * 512],
                start=True,
                stop=True,
            )
            nc.scalar.copy(G[:, n * 512 : (n + 1) * 512], gb[:])

        accs = [
            accp.tile([65, 512], dt.float32, tag=f"acc{n}", name=f"acc{n}")
            for n in range(NB)
        ]

        for t in range(T):
            at = adjp.tile([P, N], dt.float16)
            nc.sync.dma_start(out=at[:], in_=adjT[t * P : (t + 1) * P, :])
            # exp(leaky(e1+e2)) / exp(0.2*e1)  — the common exp(0.2*e1)
            # row factor cancels in the softmax divide, so the whole score
            # is max(G_i*A2_j, B2_j): ONE two-scalar tensor_scalar op.
            p0 = work.tile([P, N], dt.float16)
            nc.vector.tensor_scalar(
                out=p0[:],
                in0=G[:],
                scalar1=a2_sb[:, t : t + 1],
                scalar2=b2_sb[:, t : t + 1],
                op0=mybir.AluOpType.mult,
                op1=mybir.AluOpType.max,
            )
            p = work.tile([P, N], dt.float16)
            nc.vector.tensor_mul(p[:], p0[:], at[:])
            for n in range(NB):
                nc.tensor.matmul(
                    out=accs[n][:],
                    lhsT=whg_sb[:, t * 65 : (t + 1) * 65],
                    rhs=p[:, n * 512 : (n + 1) * 512],
                    start=(t == 0),
                    stop=(t == T - 1),
                )

        osb = singles.tile([65, N], dt.float32)
        for n in range(NB):
            nc.scalar.copy(osb[:, n * 512 : (n + 1) * 512], accs[n][:])
        for c in range(T):
            tr = ptmp.tile([P, 512], dt.float32, tag="tr", name=f"tr{c}")
            nc.tensor.transpose(tr[:, 0:65], osb[:, c * P : (c + 1) * P], ident[:])
            rec = outp.tile([P, 1], dt.float32)
            nc.vector.reciprocal(rec[:], tr[:, 64:65])
            u = outp.tile([P, F], dt.float32)
            nc.vector.tensor_scalar_mul(u[:], tr[:, 0:F], rec[:])
            mn = outp.tile([P, F], dt.float32)
            nc.vector.tensor_scalar(
                out=mn[:],
                in0=tr[:, 0:F],
                scalar1=rec[:],
                scalar2=0.0,
                op0=mybir.AluOpType.mult,
                op1=mybir.AluOpType.min,
            )
            ex = outp.tile([P, F], dt.float32)
            nc.scalar.activation(
                out=ex[:], in_=mn[:], func=mybir.ActivationFunctionType.Exp
            )
            # elu(u) = max(u, exp(min(u,0)) - 1)   (e^x >= 1+x)
            fin = outp.tile([P, F], dt.float32)
            nc.vector.scalar_tensor_tensor(
                out=fin[:],
                in0=ex[:],
                scalar=-1.0,
                in1=u[:],
                op0=mybir.AluOpType.add,
                op1=mybir.AluOpType.max,
            )
            nc.sync.dma_start(out=out[c * P : (c + 1) * P, :], in_=fin[:])

    nc.compile()
    _CACHE["nc"] = nc
    return nc


def _prep_inputs(h, adj, W, a):
    h = np.asarray(h, np.float32)
    adj = np.asarray(adj, np.float32)
    W = np.asarray(W, np.float32)
    a = np.asarray(a, np.float32)

    adjT = np.ascontiguousarray(adj.T).astype(np.float16)
    in_maps = []
    for b in range(B):
        Wh = h[b] @ W.T  # [N, F]
        e1 = Wh @ a[:F]  # [N]
        e2 = Wh @ a[F:]  # [N]
        whg = np.empty((T, P, 65), np.float32)
        whg[:, :, :F] = Wh.reshape(T, P, F)
        whg[:, :, F] = 1.0
        whg = np.ascontiguousarray(whg.transpose(1, 0, 2)).reshape(P, T * 65)
        in_maps.append(
            {
                "adjT": adjT,
                "whg": whg.astype(np.float16),
                "gr": np.exp(0.8 * e1).reshape(1, N).astype(np.float16),
                "a2b": np.ascontiguousarray(np.exp(e2).reshape(T, P).T),
                "b2b": np.ascontiguousarray(np.exp(0.2 * e2).reshape(T, P).T),
            }
        )
    return in_maps


def kernel(h, adj, W, a, _trace=False):
    nc = _build_program()
    in_maps = _prep_inputs(h, adj, W, a)
    res = run_bass_kernel_spmd(nc, in_maps, list(range(B)), trace=_trace)
    outs = np.stack([res.results[b]["out"] for b in range(B)], axis=0)
    if _trace:
        kernel.last_results = res
    return outs.astype(np.float32)



# revision 9
# speedup vs baseline: 1.0028x; 1.0028x over previous
"""GAT layer (B=8, N=2048, F=64) on 8 trn2 NeuronCores.

Strategy: data-parallel over batch B — one graph per core, adj replicated.

Math: with e = leaky_relu(e1_i + e2_j), exp(e - 0.2*e1_i) (row factor
cancels in softmax) = A2_j * max(G_i, r_j) where G = exp(0.8*e1),
A2 = exp(e2), r = exp(-0.8*e2). A2 folds into the matmul weights on the
host (whA = [Wh*A2 | A2]; row 64 yields softmax denominators), so the
device computes q_ij = max(G_i, r_j) * adj_ji and accumulates
outT[65, i] += whA_t^T @ q_t over 16 j-tiles. Divide + elu epilogue is
O(N*F) and runs on the host.

Device mapping of the N^2 elementwise stage (the bottleneck). The only
engines that can run it are DVE (fast modes need all-2-byte SBUF
operands) and the Pool/GpSimd engine (walrus only accepts mult/add
tensor_tensor there, at reduced ucode efficiency, dtype-agnostic):
  - DVE j-tiles (10): adj in fp16 (keeps DVE fast modes): score
    s = (G max r_t) via tensor_scalar in 4x mode (~0.65us), mask
    q = s * adj via tensor_tensor in 2x mode (~1.2us).
  - Pool j-tiles (6): adj in fp8e4 (halves that DMA share; Pool doesn't
    care): DVE computes the score (4x), Pool does q = s * adj
    (~4us/tile at 0.42 ucode efficiency).
  DVE ~22.5us and Pool ~24us finish together; PE accumulates ~22us.
  - G is partition-broadcast by DMA (stride-0 source AP); adj tiles are
    all SBUF-resident (no ring recycling -> few semaphore waits).
"""

import sys

import numpy as np
import ml_dtypes

for _p in ("/opt/trn_rl_repo",):
    if _p not in sys.path:
        sys.path.insert(0, _p)

from contextlib import ExitStack

import concourse.bass as bass
import concourse.tile as tile
from concourse import bacc, mybir
from concourse.bass_utils import run_bass_kernel_spmd

B, N, F = 8, 2048, 64
P = 128
T = N // P  # 16 j-tiles
NB = N // 512  # 4 psum banks of moving-free 512

# Tiles masked on the Pool engine (fp8 adj); rest on DVE (fp16 adj).
# Spread through the PE accumulation chain so PE never waits long on a
# slow Pool tile.
POOL_TILES = (1, 4, 7, 10, 13, 15)
DVE_TILES = tuple(t for t in range(T) if t not in POOL_TILES)
# adj DMA batches: (tile list, dtype) -> one DMA each, pipeline-friendly.
ADJ16_GROUPS = [DVE_TILES[0:4], DVE_TILES[4:7], DVE_TILES[7:10]]
ADJ8_GROUPS = [POOL_TILES[0:3], POOL_TILES[3:6]]

_CACHE = {}


def _build_program():
    if "nc" in _CACHE:
        return _CACHE["nc"]
    dt = mybir.dt
    nc = bacc.Bacc("TRN2", target_bir_lowering=False, debug=False)

    adj16 = nc.dram_tensor(
        "adj16", [P, len(DVE_TILES) * N], dt.float16, kind="ExternalInput"
    ).ap()
    adj8 = nc.dram_tensor(
        "adj8", [P, len(POOL_TILES) * N], dt.float8e4, kind="ExternalInput"
    ).ap()
    g = nc.dram_tensor("g", [1, N], dt.float16, kind="ExternalInput").ap()
    rsc = nc.dram_tensor("rsc", [P, T], dt.float32, kind="ExternalInput").ap()
    wha = nc.dram_tensor("wha", [P, T * 65], dt.float16, kind="ExternalInput").ap()
    outT = nc.dram_tensor("outT", [65, N], dt.float16, kind="ExternalOutput").ap()

    with tile.TileContext(nc) as tc, ExitStack() as ctx:
        singles = ctx.enter_context(tc.tile_pool(name="singles", bufs=1))
        qpool = ctx.enter_context(tc.tile_pool(name="qpool", bufs=6))
        accp = ctx.enter_context(tc.tile_pool(name="accp", bufs=1, space="PSUM"))

        g_sb = singles.tile([P, N], dt.float16)
        nc.sync.dma_start(out=g_sb[:], in_=g.to_broadcast((P, N)))
        rsc_sb = singles.tile([P, T], dt.float32)
        nc.sync.dma_start(out=rsc_sb[:], in_=rsc)
        wha_sb = singles.tile([P, T * 65], dt.float16)
        nc.sync.dma_start(out=wha_sb[:], in_=wha)

        # adj tiles, all SBUF-resident. Each group lands with one DMA.
        adj_sb = {}
        a16 = singles.tile([P, len(DVE_TILES) * N], dt.float16, name="a16")
        for gi, grp in enumerate(ADJ16_GROUPS):
            base = sum(len(g_) for g_ in ADJ16_GROUPS[:gi])
            nc.sync.dma_start(
                out=a16[:, base * N : (base + len(grp)) * N],
                in_=adj16[:, base * N : (base + len(grp)) * N],
            )
            for k, t in enumerate(grp):
                adj_sb[t] = a16[:, (base + k) * N : (base + k + 1) * N]
        a8 = singles.tile([P, len(POOL_TILES) * N], dt.float8e4, name="a8")
        for gi, grp in enumerate(ADJ8_GROUPS):
            base = sum(len(g_) for g_ in ADJ8_GROUPS[:gi])
            nc.sync.dma_start(
                out=a8[:, base * N : (base + len(grp)) * N],
                in_=adj8[:, base * N : (base + len(grp)) * N],
            )
            for k, t in enumerate(grp):
                adj_sb[t] = a8[:, (base + k) * N : (base + k + 1) * N]

        # Scores: DVE tensor_scalar in 4x mode, independent of adj DMAs.
        scores = {}
        for t in range(T):
            st = singles.tile([P, N], dt.float16, name=f"s{t}")
            nc.vector.tensor_scalar_max(st[:], g_sb[:], rsc_sb[:, t : t + 1])
            scores[t] = st

        accs = [
            accp.tile([65, 512], dt.float32, tag=f"acc{n}", name=f"acc{n}")
            for n in range(NB)
        ]

        for t in range(T):
            qt = qpool.tile([P, N], dt.float16)
            eng = nc.gpsimd if t in POOL_TILES else nc.vector
            eng.tensor_tensor(qt[:], scores[t][:], adj_sb[t], mybir.AluOpType.mult)
            for n in range(NB):
                nc.tensor.matmul(
                    out=accs[n][:],
                    lhsT=wha_sb[:, t * 65 : (t + 1) * 65],
                    rhs=qt[:, n * 512 : (n + 1) * 512],
                    start=(t == 0),
                    stop=(t == T - 1),
                )

        osb = singles.tile([65, N], dt.float16)
        for n in range(NB):
            nc.scalar.copy(osb[:, n * 512 : (n + 1) * 512], accs[n][:])
        nc.sync.dma_start(out=outT, in_=osb[:])

    nc.compile()
    _CACHE["nc"] = nc
    return nc


def _prep_inputs(h, adj, W, a):
    h = np.asarray(h, np.float32)
    adj = np.asarray(adj, np.float32)
    W = np.asarray(W, np.float32)
    a = np.asarray(a, np.float32)

    # adj^T tiles regrouped by engine: [P, ntiles*N] with tile t's rows
    # t*128+p at column block k*N (k = position in the engine's list).
    adjT = adj.T.reshape(T, P, N)
    a16 = np.ascontiguousarray(
        adjT[list(DVE_TILES)].transpose(1, 0, 2).reshape(P, len(DVE_TILES) * N)
    ).astype(np.float16)
    a8 = np.ascontiguousarray(
        adjT[list(POOL_TILES)].transpose(1, 0, 2).reshape(P, len(POOL_TILES) * N)
    ).astype(ml_dtypes.float8_e4m3)

    Wh = np.einsum("bnf,of->bno", h, W)  # [B, N, F]
    e1 = Wh @ a[:F]  # [B, N]
    e2 = Wh @ a[F:]  # [B, N]
    A2 = np.exp(e2)
    G = np.exp(0.8 * e1).astype(np.float16)  # [B, N]
    r = np.exp(-0.8 * e2).astype(np.float32)  # [B, N]
    whA = np.concatenate([Wh * A2[..., None], A2[..., None]], axis=2)  # [B, N, 65]
    whA = np.ascontiguousarray(
        whA.reshape(B, T, P, 65).transpose(0, 2, 1, 3)
    ).reshape(B, P, T * 65)

    in_maps = []
    for b in range(B):
        in_maps.append(
            {
                "adj16": a16,
                "adj8": a8,
                "g": G[b].reshape(1, N),
                "rsc": np.ascontiguousarray(r[b].reshape(T, P).T),
                "wha": whA[b].astype(np.float16),
            }
        )
    return in_maps


def kernel(h, adj, W, a, _trace=False):
    nc = _build_program()
    in_maps = _prep_inputs(h, adj, W, a)
    res = run_bass_kernel_spmd(nc, in_maps, list(range(B)), trace=_trace)
    outs = np.empty((B, N, F), np.float32)
    for b in range(B):
        outT = np.asarray(res.results[b]["outT"], dtype=np.float32)  # [65, N]
        hp = outT[:F].T / outT[F][:, None]
        outs[b] = np.where(hp > 0, hp, np.expm1(hp))
    if _trace:
        kernel.last_results = res
    return outs


# revision 10
# speedup vs baseline: 1.0998x; 1.0967x over previous
"""GAT layer (B=8, N=2048, F=64) on 8 trn2 NeuronCores.

Strategy: data-parallel over batch B — one graph per core, adj replicated.

Math: with e = leaky_relu(e1_i + e2_j), exp(e - 0.2*e1_i) (row factor
cancels in softmax) = A2_j * max(G_i, r_j) where G = exp(0.8*e1),
A2 = exp(e2), r = exp(-0.8*e2). A2 folds into the matmul weights on the
host (whA = [Wh*A2 | A2]; row 64 yields softmax denominators), so the
device computes q_ij = max(G_i, r_j) * adj_ji and accumulates
outT[65, i] += whA_t^T @ q_t over 16 j-tiles. Divide + elu epilogue is
O(N*F) and runs on the host.

The binding resource is SBUF bandwidth (~7-9 B/ns/partition aggregate,
measured): the kernel is designed to minimize SBUF traffic, not engine
cycles. The whole N^2 elementwise stage is ONE fused DVE
scalar_tensor_tensor per j-tile — q_t = (G max r_t) * adj_t — reading
G (fp16) + adj (fp8, exact for 0/1) and writing q (fp16): 10KB/partition
per tile vs 20KB for a split score+mask pipeline, and no cross-engine
handoffs (DVE -> PE only). adj tiles and q tiles are all SBUF-resident
(no ring recycling -> minimal semaphore waits); G is partition-broadcast
by DMA (stride-0 source AP).
"""

import sys

import numpy as np
import ml_dtypes

for _p in ("/opt/trn_rl_repo",):
    if _p not in sys.path:
        sys.path.insert(0, _p)

from contextlib import ExitStack

import concourse.bass as bass
import concourse.tile as tile
from concourse import bacc, mybir
from concourse.bass_utils import run_bass_kernel_spmd

B, N, F = 8, 2048, 64
P = 128
T = N // P  # 16 j-tiles
NB = N // 512  # 4 psum banks of moving-free 512
QUAD = 4  # j-tiles per adj DMA

_CACHE = {}


def _build_program():
    if "nc" in _CACHE:
        return _CACHE["nc"]
    dt = mybir.dt
    nc = bacc.Bacc("TRN2", target_bir_lowering=False, debug=False)

    adjq = nc.dram_tensor(
        "adjq", [(T // QUAD) * P, QUAD * N], dt.float8e4, kind="ExternalInput"
    ).ap()
    g = nc.dram_tensor("g", [1, N], dt.float16, kind="ExternalInput").ap()
    rsc = nc.dram_tensor("rsc", [P, T], dt.float32, kind="ExternalInput").ap()
    wha = nc.dram_tensor("wha", [P, T * 65], dt.float16, kind="ExternalInput").ap()
    outT = nc.dram_tensor("outT", [65, N], dt.float16, kind="ExternalOutput").ap()

    with tile.TileContext(nc) as tc, ExitStack() as ctx:
        singles = ctx.enter_context(tc.tile_pool(name="singles", bufs=1))
        accp = ctx.enter_context(tc.tile_pool(name="accp", bufs=1, space="PSUM"))

        g_sb = singles.tile([P, N], dt.float16)
        nc.sync.dma_start(out=g_sb[:], in_=g.to_broadcast((P, N)))
        rsc_sb = singles.tile([P, T], dt.float32)
        nc.sync.dma_start(out=rsc_sb[:], in_=rsc)
        wha_sb = singles.tile([P, T * 65], dt.float16)
        nc.sync.dma_start(out=wha_sb[:], in_=wha)

        # All 16 adj tiles SBUF-resident: 4 quads of [128, 4*N] fp8.
        adj_sb = []
        for qd in range(T // QUAD):
            at = singles.tile([P, QUAD * N], dt.float8e4, name=f"adj{qd}")
            nc.sync.dma_start(out=at[:], in_=adjq[qd * P : (qd + 1) * P, :])
            adj_sb.append(at)

        def adj_tile(t):
            return adj_sb[t // QUAD][:, (t % QUAD) * N : (t % QUAD + 1) * N]

        accs = [
            accp.tile([65, 512], dt.float32, tag=f"acc{n}", name=f"acc{n}")
            for n in range(NB)
        ]

        for t in range(T):
            qt = singles.tile([P, N], dt.float16, name=f"q{t}")
            nc.vector.scalar_tensor_tensor(
                out=qt[:],
                in0=g_sb[:],
                scalar=rsc_sb[:, t : t + 1],
                in1=adj_tile(t),
                op0=mybir.AluOpType.max,
                op1=mybir.AluOpType.mult,
            )
            for n in range(NB):
                nc.tensor.matmul(
                    out=accs[n][:],
                    lhsT=wha_sb[:, t * 65 : (t + 1) * 65],
                    rhs=qt[:, n * 512 : (n + 1) * 512],
                    start=(t == 0),
                    stop=(t == T - 1),
                )

        osb = singles.tile([65, N], dt.float16)
        for n in range(NB):
            nc.scalar.copy(osb[:, n * 512 : (n + 1) * 512], accs[n][:])
        nc.sync.dma_start(out=outT, in_=osb[:])

    nc.compile()
    _CACHE["nc"] = nc
    return nc


def _prep_inputs(h, adj, W, a):
    h = np.asarray(h, np.float32)
    adj = np.asarray(adj, np.float32)
    W = np.asarray(W, np.float32)
    a = np.asarray(a, np.float32)

    # Quad-major adj^T: row qd*128+p holds j-rows (4*qd+kk)*128+p, kk=0..3.
    adjT = adj.T.reshape(T // QUAD, QUAD, P, N).transpose(0, 2, 1, 3)
    adjq = np.ascontiguousarray(adjT.reshape((T // QUAD) * P, QUAD * N)).astype(
        ml_dtypes.float8_e4m3
    )

    Wh = np.einsum("bnf,of->bno", h, W)  # [B, N, F]
    e1 = Wh @ a[:F]  # [B, N]
    e2 = Wh @ a[F:]  # [B, N]
    A2 = np.exp(e2)
    G = np.exp(0.8 * e1).astype(np.float16)  # [B, N]
    r = np.exp(-0.8 * e2).astype(np.float32)  # [B, N]
    whA = np.concatenate([Wh * A2[..., None], A2[..., None]], axis=2)  # [B, N, 65]
    whA = np.ascontiguousarray(
        whA.reshape(B, T, P, 65).transpose(0, 2, 1, 3)
    ).reshape(B, P, T * 65)

    in_maps = []
    for b in range(B):
        in_maps.append(
            {
                "adjq": adjq,
                "g": G[b].reshape(1, N),
                "rsc": np.ascontiguousarray(r[b].reshape(T, P).T),
                "wha": whA[b].astype(np.float16),
            }
        )
    return in_maps


def kernel(h, adj, W, a, _trace=False):
    nc = _build_program()
    in_maps = _prep_inputs(h, adj, W, a)
    res = run_bass_kernel_spmd(nc, in_maps, list(range(B)), trace=_trace)
    outs = np.empty((B, N, F), np.float32)
    for b in range(B):
        outT = np.asarray(res.results[b]["outT"], dtype=np.float32)  # [65, N]
        hp = outT[:F].T / outT[F][:, None]
        outs[b] = np.where(hp > 0, hp, np.expm1(hp))
    if _trace:
        kernel.last_results = res
    return outs
